# revision 1
# baseline (speedup 1.0000x reference)
"""Trainium2 Bass kernel for nn_CombinedLoss (dice + bce + kl(soft-label blur) + mse + soft-bce).

Self-contained: hardcodes shapes [8,1,1024,1024] fp32, 8 NeuronCores.
Sharding: H axis split into 8 slabs of 128 rows per core (data parallel over rows).
The gaussian blur (sigma=2, radius 8, axes B/H/W) runs on the tensor engine as
two matmul stages in fp16 (exact 0/1 inputs, fp32 PSUM accumulation):
  stage 1: combined (batch x H) mixing  Z[w, (h',b')] = sum_{(h,b)} X[(h,b), w] * A[(h,b),(h',b')]
  stage 2: W mixing                     sm[(h',b'), w''] = sum_w' Z[w', .] * BwT[w', w'']
H halos (+-8 rows) come from host-side symmetric padding. The only cross-core
coupling is one AllGather of per-core maxes (soft-label normalization); the
scalar loss is assembled on host from per-core partial sums.
"""

import numpy as np

import concourse.bass as bass
import concourse.bacc as bacc
import concourse.tile as tile
import concourse.mybir as mybir
from concourse.bass_utils import run_bass_kernel_spmd

F32 = mybir.dt.float32
F16 = mybir.dt.float16
AF = mybir.ActivationFunctionType
ALU = mybir.AluOpType

SIGMA = 2.0
R = 8
KT = 2 * R + 1
SMOOTH = 1e-5
EPS8 = float(np.float32(1e-8))
QMIN = 1e-7
PMIN = 2.4e-4


def gauss_kernel():
    t = np.arange(-R, R + 1, dtype=np.float64)
    k = np.exp(-0.5 * (t / SIGMA) ** 2)
    k = k / k.sum()
    # reference casts taps to float32
    return k.astype(np.float32).astype(np.float64)


def blur_mat(n):
    """[n, n] float64 matrix M with (blur(x))[j] = sum_src M[j, src] x[src],
    symmetric padding, matching scipy/reference semantics."""
    k = gauss_kernel()
    I = np.eye(n, dtype=np.float64)
    P = np.pad(I, ((R, R), (0, 0)), mode="symmetric")
    M = np.zeros((n, n), dtype=np.float64)
    for j in range(n):
        M[j] = k @ P[j : j + KT, :]
    return M


def build_stage1_mats():
    """A_cat [128, 256] = [A1 | A0] combined (H-band x batch-mix) matrices."""
    k = gauss_kernel()
    Wb = blur_mat(8)  # batch mixing [out_b', src_b]
    A0 = np.zeros((128, 128), dtype=np.float64)
    A1 = np.zeros((128, 128), dtype=np.float64)
    for rho in range(16):
        for m in range(16):
            d0 = rho - m
            d1 = 16 + rho - m
            for b in range(8):
                for bp in range(8):
                    if 0 <= d0 <= 16:
                        A0[rho * 8 + b, m * 8 + bp] = k[d0] * Wb[bp, b]
                    if 0 <= d1 <= 16:
                        A1[rho * 8 + b, m * 8 + bp] = k[d1] * Wb[bp, b]
    return np.concatenate([A1, A0], axis=1)  # [128, 256]


def _bank_splits(lo, hi, extra=()):
    """Split [lo, hi) at 512-boundaries (PSUM bank) and any extra points."""
    pts = {lo, hi}
    c = (lo // 512 + 1) * 512
    while c < hi:
        pts.add(c)
        c += 512
    for e in extra:
        if lo < e < hi:
            pts.add(e)
    s = sorted(pts)
    return list(zip(s[:-1], s[1:]))


def build_kernel(B=8, H=1024, W=1024, NCORES=8, use_collective=True):
    import os
    stages = set(int(s) for s in
                 os.environ.get("BISECT_STAGES", "1,2,3,5,6").split(","))
    assert B == 8
    HL = H // NCORES          # rows per core
    CH = HL // 16             # 16-row chunks per core
    T = CH + 1                # input tiles incl. halo
    G = W // 128              # w-groups
    FD = CH * W               # free dim of [128, FD] elementwise tensors

    A_cat = build_stage1_mats().astype(np.float16)
    BwT = blur_mat(W).T  # [src w', out w'']
    win = [(max(0, 128 * g - R), min(W, 128 * g + 128 + R)) for g in range(G)]
    Bwg_np = [np.ascontiguousarray(
        BwT[128 * g : 128 * g + 128, lo_c:hi_c].astype(np.float16))
        for g, (lo_c, hi_c) in enumerate(win)]

    nc = bacc.Bacc("TRN2", target_bir_lowering=False, debug=False,
                   enable_asserts=False, num_devices=NCORES)

    # ---------------- I/O (host pre-transposed: partition p = r*8 + b) ------
    targ_d = nc.dram_tensor("targ", [128, T * W], F16, kind="ExternalInput").ap()
    tstd_d = nc.dram_tensor("tstd", [128, FD], F16, kind="ExternalInput").ap()
    pred_d = nc.dram_tensor("pred", [128, FD], F16, kind="ExternalInput").ap()
    spred_d = nc.dram_tensor("spred", [128, FD], F16, kind="ExternalInput").ap()

    out_names = ["acc_pt", "acc_xt", "acc_dd", "acc_sp", "acc_sp2", "acc_smp",
                 "acc_smlog", "acc_h0", "acc_pp", "acc_max"]
    outs = {n: nc.dram_tensor(n, [128, 1], F32, kind="ExternalOutput").ap()
            for n in out_names}
    ssum_d = nc.dram_tensor("acc_ssum", [128, CH], F32, kind="ExternalOutput").ap()

    # single merged constant blob [128, ncols] fp16: [Acat | Bw windows]
    blob_parts = [A_cat] + Bwg_np
    blob = np.concatenate([np.asarray(p) for p in blob_parts], axis=1)
    blob_off = np.cumsum([0] + [p.shape[1] for p in blob_parts])
    blob_d = nc.inline_tensor(np.ascontiguousarray(blob), "constblob").ap()

    with tile.TileContext(nc) as tc:
        with (
            tc.tile_pool(name="const", bufs=1) as cpool,
            tc.tile_pool(name="big", bufs=1) as bpool,
            tc.tile_pool(name="zpool", bufs=1) as zpool,
            tc.tile_pool(name="acc", bufs=1) as apool,
            tc.tile_pool(name="ps1", bufs=4, space="PSUM") as ps1,
            tc.tile_pool(name="dram", bufs=1, space="DRAM") as dpool,
        ):
            CONST = cpool.tile([128, int(blob_off[-1])], F16, tag="CONST")

            def bpart(i):
                return CONST[:, int(blob_off[i]) : int(blob_off[i + 1])]

            Acat = bpart(0)
            Bwg = [bpart(1 + g) for g in range(G)]

            # ---------------- input DMAs (SP queue order) ----------------
            x_sb = bpool.tile([128, FD], F16, tag="x_sb")
            nc.sync.dma_start(out=x_sb[:], in_=pred_d)
            Xs = bpool.tile([128, T * W], F16, tag="Xs")
            nc.sync.dma_start(out=Xs[:], in_=targ_d)
            nc.sync.dma_start(out=CONST[:], in_=blob_d)
            ts_sb = bpool.tile([128, FD], F16, tag="ts_sb")
            nc.sync.dma_start(out=ts_sb[:], in_=tstd_d)
            s_sb = bpool.tile([128, FD], F16, tag="s_sb")
            nc.sync.dma_start(out=s_sb[:], in_=spred_d)

            accs = {n: apool.tile([128, 1], F32, name=f"t_{n}", tag=f"t_{n}")
                    for n in out_names}
            for n in out_names:
                nc.gpsimd.memset(accs[n][:], 0.0)

            scrA = bpool.tile([128, FD], F16, tag="scrA")  # ACT dump
            scrD = bpool.tile([128, FD], F16, tag="scrD")  # DVE dump
            d_sb = bpool.tile([128, FD], F16, tag="d_sb")

            # ---------------- sigma first on ACT ----------------
            p_sb = bpool.tile([128, FD], F16, tag="p_sb")
            nc.scalar.activation(p_sb[:], x_sb[:], AF.Sigmoid)

            # r-clamp for S_sp on DVE early (d_sb scratch)
            nc.vector.tensor_scalar(d_sb[:], p_sb[:], 1.0, -PMIN,
                                    ALU.subtract, ALU.min)

            # ---------------- stage 1: (B x H) mix ----------------
            Zb = []
            for g in range(G if 2 in stages else 0):
                zp = ps1.tile([128, CH * 128], F32, name=f"zp{g}", tag="zp")
                mms = []
                for t in range(T):
                    co0 = 0 if t == 0 else 128 * (t - 1)
                    co1 = 128 * t if t == T - 1 else 128 * (t + 1)
                    for (a, bcol) in _bank_splits(co0, co1, extra=(128 * t,)):
                        mms.append((t, a, bcol, a - 128 * (t - 1),
                                    bcol - 128 * (t - 1)))
                first_bank = set()
                n_mm = 0
                for (t, a, bcol, ra, rb) in mms:
                    bank = a // 512
                    st = bank not in first_bank
                    first_bank.add(bank)
                    n_mm += 1
                    nc.tensor.matmul(
                        zp[:, a:bcol],
                        Xs[:, t * W + 128 * g : t * W + 128 * g + 128],
                        Acat[:, ra:rb],
                        start=st, stop=(n_mm == len(mms)),
                        skip_group_check=True)
                zb = zpool.tile([128, CH * 128], F16, name=f"zb{g}", tag=f"zb{g}")
                if g % 2 == 0:
                    nc.scalar.copy(zb[:], zp[:])
                else:
                    nc.vector.tensor_copy(zb[:], zp[:])
                Zb.append(zb)

            # ---------------- pred-side products on DVE ----------------
            if 1 in stages:
                nc.vector.tensor_mul(scrD[:], p_sb[:], ts_sb[:])
                nc.vector.tensor_scalar(scrD[:], scrD[:], 1.0, 0.0, ALU.mult,
                                        ALU.add, accum_out=accs["acc_pt"][:])
                nc.vector.tensor_mul(scrD[:], x_sb[:], ts_sb[:])
                nc.vector.tensor_scalar(scrD[:], scrD[:], 1.0, 0.0, ALU.mult,
                                        ALU.add, accum_out=accs["acc_xt"][:])
                dx_sb = bpool.tile([128, FD], F16, tag="ts_sb", name="dx_sb")
                nc.vector.tensor_sub(dx_sb[:], x_sb[:], s_sb[:])

                # ACT chain (natural_log_exp set stays loaded from here on)
                biasm1 = apool.tile([128, 1], F32, tag="biasm1")
                nc.gpsimd.memset(biasm1[:], -1.0)
                e2 = bpool.tile([128, FD], F16, tag="lsm", name="e2")
                nc.scalar.activation(e2[:], p_sb[:], AF.Exp, bias=biasm1[:],
                                     scale=2.0)
                nc.scalar.activation(scrA[:], e2[:], AF.Ln, bias=1.0,
                                     accum_out=accs["acc_sp2"][:])
                nc.scalar.activation(scrA[:], p_sb[:], AF.Square,
                                     accum_out=accs["acc_pp"][:])
                nc.scalar.activation(scrA[:], dx_sb[:], AF.Square,
                                     accum_out=accs["acc_dd"][:])
                nc.scalar.activation(scrA[:], d_sb[:], AF.Ln, scale=-1.0,
                                     accum_out=accs["acc_sp"][:])

            # ---------------- stage 2 + pipelined sm consumers ------------
            sm_bf = bpool.tile([128, FD], F16, tag="sm_bf")
            maxt = apool.tile([128, CH], F32, tag="maxt")
            ssumt = apool.tile([128, CH], F32, tag="ssumt")
            lsm = bpool.tile([128, FD], F16, tag="lsm", name="lsm")
            smph = apool.tile([128, 2], F32, tag="smph")
            smlh = apool.tile([128, 2], F32, tag="smlh")
            HH2 = FD // 2

            def sm_half(hh):
                sl = slice(hh * HH2, (hh + 1) * HH2)
                nc.scalar.activation(lsm[:, sl], sm_bf[:, sl], AF.Ln)
                nc.vector.tensor_mul(scrD[:, sl], sm_bf[:, sl], p_sb[:, sl])
                nc.vector.tensor_scalar(scrD[:, sl], scrD[:, sl], 1.0, 0.0,
                                        ALU.mult, ALU.add,
                                        accum_out=smph[:, hh : hh + 1])
                nc.vector.tensor_mul(scrD[:, sl], sm_bf[:, sl], lsm[:, sl])
                nc.vector.tensor_scalar(scrD[:, sl], scrD[:, sl], 1.0, 0.0,
                                        ALU.mult, ALU.add,
                                        accum_out=smlh[:, hh : hh + 1])

            for c in range(CH if 3 in stages else 0):
                sp = ps1.tile([128, W], F32, name=f"smp{c}", tag="zp")
                mms = []
                for g in range(G):
                    for (a, bcol) in _bank_splits(*win[g]):
                        mms.append((g, a, bcol))
                first_bank = set()
                total = 0
                for (g, a, bcol) in mms:
                    lo_c = win[g][0]
                    bank = a // 512
                    st = bank not in first_bank
                    first_bank.add(bank)
                    total += 1
                    nc.tensor.matmul(
                        sp[:, a:bcol],
                        Zb[g][:, 128 * c : 128 * c + 128],
                        Bwg[g][:, a - lo_c : bcol - lo_c],
                        start=st, stop=(total == len(mms)),
                        skip_group_check=True)
                # psum -> sbuf fp16 copy with fused sum accumulation
                nc.vector.tensor_scalar(sm_bf[:, c * W : (c + 1) * W], sp[:],
                                        1.0, 0.0, ALU.mult, ALU.add,
                                        accum_out=ssumt[:, c : c + 1])
                nc.vector.reduce_max(maxt[:, c : c + 1], sp[:],
                                     mybir.AxisListType.X)
                if 3 in stages and c == max(0, CH // 2 - 1):
                    sm_half(0)
            if 3 in stages:
                sm_half(1)
                nc.vector.tensor_scalar(accs["acc_smp"][:], smph[:, 0:1], 1.0,
                                        smph[:, 1:2], ALU.mult, ALU.add)
                nc.vector.tensor_scalar(accs["acc_smlog"][:], smlh[:, 0:1], 1.0,
                                        smlh[:, 1:2], ALU.mult, ALU.add)
            nc.sync.dma_start(out=ssum_d, in_=ssumt[:])

            # ---------------- global max + M ----------------
            if 5 in stages:
                maxfin = apool.tile([128, 1], F32, tag="maxfin")
                nc.vector.reduce_max(maxfin[:], maxt[:], mybir.AxisListType.X)
                nc.vector.tensor_copy(accs["acc_max"][:], maxfin[:])
                mrow = apool.tile([1, NCORES * 128], F32, tag="mrow")
                if use_collective:
                    cc_in = dpool.tile([128, 1], F32, tag="cc_in")
                    cc_out = dpool.tile([1, NCORES * 128], F32,
                                        addr_space="Shared", tag="cc_out")
                    nc.sync.dma_start(out=cc_in[:], in_=maxfin[:])
                    nc.gpsimd.collective_compute(
                        "AllGather", ALU.bypass,
                        replica_groups=[list(range(NCORES))],
                        ins=[cc_in[:]], outs=[cc_out[:]])
                    nc.sync.dma_start(out=mrow[:], in_=cc_out[:])
                else:
                    cc_in = dpool.tile([1, 128], F32, tag="cc_in")
                    nc.sync.dma_start(out=cc_in[:], in_=maxfin[:])
                    for rep in range(NCORES):
                        nc.sync.dma_start(
                            out=mrow[:, rep * 128 : (rep + 1) * 128],
                            in_=cc_in[:])
                m1 = apool.tile([1, 1], F32, tag="m1")
                nc.vector.reduce_max(m1[:], mrow[:], mybir.AxisListType.X)
                m1e = apool.tile([1, 1], F32, tag="m1e")
                nc.vector.tensor_scalar(m1e[:], m1[:], EPS8, None, ALU.add)
                ones1 = apool.tile([1, 128], F32, tag="ones1")
                nc.gpsimd.memset(ones1[:], 1.0)
                M_ps = ps1.tile([128, 1], F32, tag="zp", name="M_ps")
                nc.tensor.matmul(M_ps[:], ones1[:], m1e[:], start=True, stop=True,
                                 skip_group_check=True)
                M_ap = apool.tile([128, 1], F32, tag="M_ap")
                nc.vector.tensor_copy(M_ap[:], M_ps[:])

            # ---------------- post-sync (pipelined halves) ----------------
            if 6 in stages:
                lms = bpool.tile([128, FD], F16, tag="x_sb", name="lms")
                h0h = apool.tile([128, 2], F32, tag="h0h")
                for hh in range(2):
                    sl = slice(hh * HH2, (hh + 1) * HH2)
                    nc.vector.tensor_scalar(scrD[:, sl], sm_bf[:, sl], M_ap[:],
                                            -QMIN, ALU.subtract, ALU.min)
                    nc.scalar.activation(lms[:, sl], scrD[:, sl], AF.Ln,
                                         scale=-1.0)
                    nc.vector.tensor_mul(d_sb[:, sl], scrD[:, sl], lms[:, sl])
                    nc.vector.tensor_scalar(d_sb[:, sl], d_sb[:, sl], 1.0, 0.0,
                                            ALU.mult, ALU.add,
                                            accum_out=h0h[:, hh : hh + 1])
                nc.vector.tensor_scalar(accs["acc_h0"][:], h0h[:, 0:1], 1.0,
                                        h0h[:, 1:2], ALU.mult, ALU.add)

            # ---------------- outputs ----------------
            for n in out_names:
                nc.sync.dma_start(out=outs[n], in_=accs[n][:])

    nc.compile()
    meta = dict(B=B, H=H, W=W, NCORES=NCORES, HL=HL, CH=CH, G=G, FD=FD,
                out_names=out_names)
    return nc, meta


# ---------------------------------------------------------------------------
_CACHE = {}


def _get_built(key=(8, 1024, 1024, 8)):
    if key not in _CACHE:
        _CACHE[key] = build_kernel(*key)
    return _CACHE[key]


def _to_tiles(slab):
    """[8, HH, W] -> [128, (HH/16)*W] with partition p = r*8+b, free (t, w)."""
    B, HH, W = slab.shape
    T = HH // 16
    a = slab.reshape(B, T, 16, W).transpose(2, 0, 1, 3)  # [16, 8, T, W]
    return np.ascontiguousarray(a).reshape(128, T * W)


def make_in_maps(target, pred, spred, NCORES, HL):
    B = target.shape[0]
    H, W = target.shape[-2], target.shape[-1]
    t2 = np.asarray(target, dtype=np.float32).reshape(B, H, W).astype(np.float16)
    tpad = np.pad(t2, ((0, 0), (R, R), (0, 0)), mode="symmetric")
    p2 = np.asarray(pred, dtype=np.float32).reshape(B, H, W).astype(np.float16)
    s2 = np.asarray(spred, dtype=np.float32).reshape(B, H, W).astype(np.float16)
    in_maps = []
    for i in range(NCORES):
        in_maps.append({
            "targ": _to_tiles(tpad[:, i * HL : i * HL + HL + 2 * R, :]),
            "tstd": _to_tiles(t2[:, i * HL : (i + 1) * HL, :]),
            "pred": _to_tiles(p2[:, i * HL : (i + 1) * HL, :]),
            "spred": _to_tiles(s2[:, i * HL : (i + 1) * HL, :]),
        })
    return in_maps


def host_t_sums(target):
    """Exact per-batch sums of the binary target (host side, fp64)."""
    B = target.shape[0]
    return np.asarray(target, dtype=np.float64).reshape(B, -1).sum(axis=1)


def assemble(results, meta, n_tot, t_b, return_parts=False):
    out_names = meta["out_names"]
    NC = meta["NCORES"]
    acc = {n: np.stack([results[i][n].astype(np.float64).reshape(128)
                        for i in range(NC)]) for n in out_names}
    ssum = np.stack([results[i]["acc_ssum"].astype(np.float64).reshape(-1)
                     for i in range(NC)])

    per_b = lambda a: a.reshape(NC, 16, 8).sum(axis=(0, 1))  # noqa: E731
    tot = lambda a: float(a.sum())  # noqa: E731

    pt_b = per_b(acc["acc_pt"])
    pp_b = per_b(acc["acc_pp"])
    S_sp = -tot(acc["acc_sp"])  # device accumulates ln(1-p) = -softplus(x)
    S_xt = tot(acc["acc_xt"])
    S_dd = tot(acc["acc_dd"])
    S_sp2 = tot(acc["acc_sp2"])
    S_smp = tot(acc["acc_smp"])
    S_smlog = tot(acc["acc_smlog"])
    S_h0 = tot(acc["acc_h0"])
    S_sm = float(ssum.sum())
    t_cnt = float(t_b.sum())

    mx = float(acc["acc_max"].max())
    M = float(np.float32(mx) + np.float32(EPS8))

    dice = float(np.mean(1.0 - (2.0 * pt_b + SMOOTH) / (pp_b + t_b + SMOOTH)))
    bce = (S_sp - S_xt) / n_tot

    if mx < 1e-8:
        kl = 0.0
    else:
        lnM = np.log(M)
        sum_t1_ln = (S_smlog - lnM * S_sm) / M
        sum_t0_ln = ((-S_h0) - lnM * (n_tot * M - S_sm)) / M
        sum_t1u = (S_sm - 2.0 * S_smp) / M
        kl = (sum_t1_ln + sum_t0_ln + S_sp2 + sum_t1u) / n_tot
        kl = min(max(kl, 0.0), 2.0)

    mxm = 1.0 if t_cnt > 0 else 0.0
    mnm = 0.0 if t_cnt < n_tot else 1.0
    if mnm == 1.0 or mxm == 0.0:
        S_xts = 0.0
    else:
        S_xts = S_xt / (mxm - mnm + float(np.float32(1e-8)))
    bsoft = (S_sp - S_xts) / n_tot

    div = S_dd / n_tot

    lam = np.array([1.0, 1.0, 0.5, 0.5, 0.5])
    lam = lam / lam.sum()
    out = lam[0] * dice + lam[1] * bce + lam[2] * kl + lam[3] * div + lam[4] * bsoft
    if return_parts:
        return np.float32(out), dict(dice=dice, bce=bce, kl=kl, div=div,
                                     bsoft=bsoft, mx=mx, S_sm=S_sm, S_smp=S_smp,
                                     S_smlog=S_smlog, S_h0=-S_h0, S_sp=S_sp,
                                     S_sp2=S_sp2, S_xt=S_xt, S_dd=S_dd)
    return np.asarray(out, dtype=np.float32).reshape(())


def kernel(image, pred, target, second_pred):
    nc, meta = _get_built()
    in_maps = make_in_maps(target, pred, second_pred, meta["NCORES"], meta["HL"])
    res = run_bass_kernel_spmd(nc, in_maps, core_ids=list(range(meta["NCORES"])))
    n_tot = float(np.prod(target.shape))
    return assemble(res.results, meta, n_tot, host_t_sums(target))



# revision 3
# speedup vs baseline: 12.6885x; 12.6885x over previous
"""Trainium2 Bass kernel for nn_CombinedLoss (dice + bce + kl(soft-label blur) + mse + soft-bce).

Self-contained: hardcodes shapes [8,1,1024,1024] fp32, 8 NeuronCores.
Sharding: H axis split into 8 slabs of 128 rows per core (data parallel over rows).
The gaussian blur (sigma=2, radius 8, axes B/H/W) runs on the tensor engine as
two matmul stages in fp16 (exact 0/1 inputs, fp32 PSUM accumulation):
  stage 1: combined (batch x H) mixing  Z[w, (h',b')] = sum_{(h,b)} X[(h,b), w] * A[(h,b),(h',b')]
  stage 2: W mixing                     sm[(h',b'), w''] = sum_w' Z[w', .] * BwT[w', w'']
H halos (+-8 rows) come from host-side symmetric padding. The only cross-core
coupling is one AllGather of per-core maxes (soft-label normalization); the
scalar loss is assembled on host from per-core partial sums.
"""

import numpy as np

import concourse.bass as bass
import concourse.bacc as bacc
import concourse.tile as tile
import concourse.mybir as mybir
from concourse.bass_utils import run_bass_kernel_spmd

F32 = mybir.dt.float32
F16 = mybir.dt.float16
AF = mybir.ActivationFunctionType
ALU = mybir.AluOpType

SIGMA = 2.0
R = 8
KT = 2 * R + 1
SMOOTH = 1e-5
EPS8 = float(np.float32(1e-8))
QMIN = 1e-7
PMIN = 2.4e-4


def gauss_kernel():
    t = np.arange(-R, R + 1, dtype=np.float64)
    k = np.exp(-0.5 * (t / SIGMA) ** 2)
    k = k / k.sum()
    # reference casts taps to float32
    return k.astype(np.float32).astype(np.float64)


def blur_mat(n):
    """[n, n] float64 matrix M with (blur(x))[j] = sum_src M[j, src] x[src],
    symmetric padding, matching scipy/reference semantics."""
    k = gauss_kernel()
    I = np.eye(n, dtype=np.float64)
    P = np.pad(I, ((R, R), (0, 0)), mode="symmetric")
    M = np.zeros((n, n), dtype=np.float64)
    for j in range(n):
        M[j] = k @ P[j : j + KT, :]
    return M


def build_stage1_mats():
    """A_cat [128, 256] = [A1 | A0] combined (H-band x batch-mix) matrices."""
    k = gauss_kernel()
    Wb = blur_mat(8)  # batch mixing [out_b', src_b]
    A0 = np.zeros((128, 128), dtype=np.float64)
    A1 = np.zeros((128, 128), dtype=np.float64)
    for rho in range(16):
        for m in range(16):
            d0 = rho - m
            d1 = 16 + rho - m
            for b in range(8):
                for bp in range(8):
                    if 0 <= d0 <= 16:
                        A0[rho * 8 + b, m * 8 + bp] = k[d0] * Wb[bp, b]
                    if 0 <= d1 <= 16:
                        A1[rho * 8 + b, m * 8 + bp] = k[d1] * Wb[bp, b]
    return np.concatenate([A1, A0], axis=1)  # [128, 256]


def _bank_splits(lo, hi, extra=()):
    """Split [lo, hi) at 512-boundaries (PSUM bank) and any extra points."""
    pts = {lo, hi}
    c = (lo // 512 + 1) * 512
    while c < hi:
        pts.add(c)
        c += 512
    for e in extra:
        if lo < e < hi:
            pts.add(e)
    s = sorted(pts)
    return list(zip(s[:-1], s[1:]))


def build_kernel(B=8, H=1024, W=1024, NCORES=8, use_collective=True):
    import os
    stages = set(int(s) for s in
                 os.environ.get("BISECT_STAGES", "1,2,3,5,6").split(","))
    assert B == 8
    HL = H // NCORES          # rows per core
    CH = HL // 16             # 16-row chunks per core
    T = CH + 1                # input tiles incl. halo
    G = W // 128              # w-groups
    FD = CH * W               # free dim of [128, FD] elementwise tensors

    A_cat = build_stage1_mats().astype(np.float16)
    BwT = blur_mat(W).T  # [src w', out w'']
    win = [(max(0, 128 * g - R), min(W, 128 * g + 128 + R)) for g in range(G)]
    Bwg_np = [np.ascontiguousarray(
        BwT[128 * g : 128 * g + 128, lo_c:hi_c].astype(np.float16))
        for g, (lo_c, hi_c) in enumerate(win)]

    nc = bacc.Bacc("TRN2", target_bir_lowering=False, debug=False,
                   enable_asserts=False, num_devices=NCORES)

    # ---------------- I/O (host pre-transposed: partition p = r*8 + b) ------
    targ_d = nc.dram_tensor("targ", [128, T * W], F16, kind="ExternalInput").ap()
    tstd_d = nc.dram_tensor("tstd", [128, FD], F16, kind="ExternalInput").ap()
    pred_d = nc.dram_tensor("pred", [128, FD], F16, kind="ExternalInput").ap()
    spred_d = nc.dram_tensor("spred", [128, FD], F16, kind="ExternalInput").ap()

    out_names = ["acc_pt", "acc_xt", "acc_dd", "acc_sp", "acc_sp2", "acc_smp",
                 "acc_smlog", "acc_h0", "acc_pp", "acc_max"]
    # single packed output: one ExternalOutput per core costs one axon RTT per
    # call (~85ms each), so everything lands in one [128, 10+CH] tensor:
    # columns 0..9 = the 10 accumulators, columns 10..10+CH = ssum.
    outp_d = nc.dram_tensor("outp", [128, len(out_names) + CH], F32,
                            kind="ExternalOutput").ap()
    outs = {n: outp_d[:, i : i + 1] for i, n in enumerate(out_names)}
    ssum_d = outp_d[:, len(out_names) : len(out_names) + CH]

    # single merged constant blob [128, ncols] fp16: [Acat | Bw windows]
    blob_parts = [A_cat] + Bwg_np
    blob = np.concatenate([np.asarray(p) for p in blob_parts], axis=1)
    blob_off = np.cumsum([0] + [p.shape[1] for p in blob_parts])
    blob_d = nc.inline_tensor(np.ascontiguousarray(blob), "constblob").ap()

    with tile.TileContext(nc) as tc:
        with (
            tc.tile_pool(name="const", bufs=1) as cpool,
            tc.tile_pool(name="big", bufs=1) as bpool,
            tc.tile_pool(name="zpool", bufs=1) as zpool,
            tc.tile_pool(name="acc", bufs=1) as apool,
            tc.tile_pool(name="ps1", bufs=4, space="PSUM") as ps1,
            tc.tile_pool(name="dram", bufs=1, space="DRAM") as dpool,
        ):
            CONST = cpool.tile([128, int(blob_off[-1])], F16, tag="CONST")

            def bpart(i):
                return CONST[:, int(blob_off[i]) : int(blob_off[i + 1])]

            Acat = bpart(0)
            Bwg = [bpart(1 + g) for g in range(G)]

            # ---------------- input DMAs (SP queue order) ----------------
            x_sb = bpool.tile([128, FD], F16, tag="x_sb")
            nc.sync.dma_start(out=x_sb[:], in_=pred_d)
            Xs = bpool.tile([128, T * W], F16, tag="Xs")
            nc.sync.dma_start(out=Xs[:], in_=targ_d)
            nc.sync.dma_start(out=CONST[:], in_=blob_d)
            ts_sb = bpool.tile([128, FD], F16, tag="ts_sb")
            nc.sync.dma_start(out=ts_sb[:], in_=tstd_d)
            s_sb = bpool.tile([128, FD], F16, tag="s_sb")
            nc.sync.dma_start(out=s_sb[:], in_=spred_d)

            accs = {n: apool.tile([128, 1], F32, name=f"t_{n}", tag=f"t_{n}")
                    for n in out_names}
            for n in out_names:
                nc.gpsimd.memset(accs[n][:], 0.0)

            scrA = bpool.tile([128, FD], F16, tag="scrA")  # ACT dump
            scrD = bpool.tile([128, FD], F16, tag="scrD")  # DVE dump
            d_sb = bpool.tile([128, FD], F16, tag="d_sb")

            # ---------------- sigma first on ACT ----------------
            p_sb = bpool.tile([128, FD], F16, tag="p_sb")
            nc.scalar.activation(p_sb[:], x_sb[:], AF.Sigmoid)

            # r-clamp for S_sp on DVE early (d_sb scratch)
            nc.vector.tensor_scalar(d_sb[:], p_sb[:], 1.0, -PMIN,
                                    ALU.subtract, ALU.min)

            # ---------------- stage 1: (B x H) mix ----------------
            Zb = []
            for g in range(G if 2 in stages else 0):
                zp = ps1.tile([128, CH * 128], F32, name=f"zp{g}", tag="zp")
                mms = []
                for t in range(T):
                    co0 = 0 if t == 0 else 128 * (t - 1)
                    co1 = 128 * t if t == T - 1 else 128 * (t + 1)
                    for (a, bcol) in _bank_splits(co0, co1, extra=(128 * t,)):
                        mms.append((t, a, bcol, a - 128 * (t - 1),
                                    bcol - 128 * (t - 1)))
                first_bank = set()
                n_mm = 0
                for (t, a, bcol, ra, rb) in mms:
                    bank = a // 512
                    st = bank not in first_bank
                    first_bank.add(bank)
                    n_mm += 1
                    nc.tensor.matmul(
                        zp[:, a:bcol],
                        Xs[:, t * W + 128 * g : t * W + 128 * g + 128],
                        Acat[:, ra:rb],
                        start=st, stop=(n_mm == len(mms)),
                        skip_group_check=True)
                zb = zpool.tile([128, CH * 128], F16, name=f"zb{g}", tag=f"zb{g}")
                if g % 2 == 0:
                    nc.scalar.copy(zb[:], zp[:])
                else:
                    nc.vector.tensor_copy(zb[:], zp[:])
                Zb.append(zb)

            # ---------------- pred-side products on DVE ----------------
            if 1 in stages:
                nc.vector.tensor_mul(scrD[:], p_sb[:], ts_sb[:])
                nc.vector.tensor_scalar(scrD[:], scrD[:], 1.0, 0.0, ALU.mult,
                                        ALU.add, accum_out=accs["acc_pt"][:])
                nc.vector.tensor_mul(scrD[:], x_sb[:], ts_sb[:])
                nc.vector.tensor_scalar(scrD[:], scrD[:], 1.0, 0.0, ALU.mult,
                                        ALU.add, accum_out=accs["acc_xt"][:])
                dx_sb = bpool.tile([128, FD], F16, tag="ts_sb", name="dx_sb")
                nc.vector.tensor_sub(dx_sb[:], x_sb[:], s_sb[:])

                # ACT chain (natural_log_exp set stays loaded from here on)
                biasm1 = apool.tile([128, 1], F32, tag="biasm1")
                nc.gpsimd.memset(biasm1[:], -1.0)
                e2 = bpool.tile([128, FD], F16, tag="lsm", name="e2")
                nc.scalar.activation(e2[:], p_sb[:], AF.Exp, bias=biasm1[:],
                                     scale=2.0)
                nc.scalar.activation(scrA[:], e2[:], AF.Ln, bias=1.0,
                                     accum_out=accs["acc_sp2"][:])
                nc.scalar.activation(scrA[:], p_sb[:], AF.Square,
                                     accum_out=accs["acc_pp"][:])
                nc.scalar.activation(scrA[:], dx_sb[:], AF.Square,
                                     accum_out=accs["acc_dd"][:])
                nc.scalar.activation(scrA[:], d_sb[:], AF.Ln, scale=-1.0,
                                     accum_out=accs["acc_sp"][:])

            # ---------------- stage 2 + pipelined sm consumers ------------
            sm_bf = bpool.tile([128, FD], F16, tag="sm_bf")
            maxt = apool.tile([128, CH], F32, tag="maxt")
            ssumt = apool.tile([128, CH], F32, tag="ssumt")
            lsm = bpool.tile([128, FD], F16, tag="lsm", name="lsm")
            smph = apool.tile([128, 2], F32, tag="smph")
            smlh = apool.tile([128, 2], F32, tag="smlh")
            HH2 = FD // 2

            def sm_half(hh):
                sl = slice(hh * HH2, (hh + 1) * HH2)
                nc.scalar.activation(lsm[:, sl], sm_bf[:, sl], AF.Ln)
                nc.vector.tensor_mul(scrD[:, sl], sm_bf[:, sl], p_sb[:, sl])
                nc.vector.tensor_scalar(scrD[:, sl], scrD[:, sl], 1.0, 0.0,
                                        ALU.mult, ALU.add,
                                        accum_out=smph[:, hh : hh + 1])
                nc.vector.tensor_mul(scrD[:, sl], sm_bf[:, sl], lsm[:, sl])
                nc.vector.tensor_scalar(scrD[:, sl], scrD[:, sl], 1.0, 0.0,
                                        ALU.mult, ALU.add,
                                        accum_out=smlh[:, hh : hh + 1])

            for c in range(CH if 3 in stages else 0):
                sp = ps1.tile([128, W], F32, name=f"smp{c}", tag="zp")
                mms = []
                for g in range(G):
                    for (a, bcol) in _bank_splits(*win[g]):
                        mms.append((g, a, bcol))
                first_bank = set()
                total = 0
                for (g, a, bcol) in mms:
                    lo_c = win[g][0]
                    bank = a // 512
                    st = bank not in first_bank
                    first_bank.add(bank)
                    total += 1
                    nc.tensor.matmul(
                        sp[:, a:bcol],
                        Zb[g][:, 128 * c : 128 * c + 128],
                        Bwg[g][:, a - lo_c : bcol - lo_c],
                        start=st, stop=(total == len(mms)),
                        skip_group_check=True)
                # psum -> sbuf fp16 copy with fused sum accumulation
                nc.vector.tensor_scalar(sm_bf[:, c * W : (c + 1) * W], sp[:],
                                        1.0, 0.0, ALU.mult, ALU.add,
                                        accum_out=ssumt[:, c : c + 1])
                nc.vector.reduce_max(maxt[:, c : c + 1], sp[:],
                                     mybir.AxisListType.X)
                if 3 in stages and c == max(0, CH // 2 - 1):
                    sm_half(0)
            if 3 in stages:
                sm_half(1)
                nc.vector.tensor_scalar(accs["acc_smp"][:], smph[:, 0:1], 1.0,
                                        smph[:, 1:2], ALU.mult, ALU.add)
                nc.vector.tensor_scalar(accs["acc_smlog"][:], smlh[:, 0:1], 1.0,
                                        smlh[:, 1:2], ALU.mult, ALU.add)
            nc.sync.dma_start(out=ssum_d, in_=ssumt[:])

            # ---------------- global max + M ----------------
            if 5 in stages:
                maxfin = apool.tile([128, 1], F32, tag="maxfin")
                nc.vector.reduce_max(maxfin[:], maxt[:], mybir.AxisListType.X)
                nc.vector.tensor_copy(accs["acc_max"][:], maxfin[:])
                mrow = apool.tile([1, NCORES * 128], F32, tag="mrow")
                if use_collective:
                    cc_in = dpool.tile([128, 1], F32, tag="cc_in")
                    cc_out = dpool.tile([1, NCORES * 128], F32,
                                        addr_space="Shared", tag="cc_out")
                    nc.sync.dma_start(out=cc_in[:], in_=maxfin[:])
                    nc.gpsimd.collective_compute(
                        "AllGather", ALU.bypass,
                        replica_groups=[list(range(NCORES))],
                        ins=[cc_in[:]], outs=[cc_out[:]])
                    nc.sync.dma_start(out=mrow[:], in_=cc_out[:])
                else:
                    cc_in = dpool.tile([1, 128], F32, tag="cc_in")
                    nc.sync.dma_start(out=cc_in[:], in_=maxfin[:])
                    for rep in range(NCORES):
                        nc.sync.dma_start(
                            out=mrow[:, rep * 128 : (rep + 1) * 128],
                            in_=cc_in[:])
                m1 = apool.tile([1, 1], F32, tag="m1")
                nc.vector.reduce_max(m1[:], mrow[:], mybir.AxisListType.X)
                m1e = apool.tile([1, 1], F32, tag="m1e")
                nc.vector.tensor_scalar(m1e[:], m1[:], EPS8, None, ALU.add)
                ones1 = apool.tile([1, 128], F32, tag="ones1")
                nc.gpsimd.memset(ones1[:], 1.0)
                M_ps = ps1.tile([128, 1], F32, tag="zp", name="M_ps")
                nc.tensor.matmul(M_ps[:], ones1[:], m1e[:], start=True, stop=True,
                                 skip_group_check=True)
                M_ap = apool.tile([128, 1], F32, tag="M_ap")
                nc.vector.tensor_copy(M_ap[:], M_ps[:])

            # ---------------- post-sync (pipelined halves) ----------------
            if 6 in stages:
                lms = bpool.tile([128, FD], F16, tag="x_sb", name="lms")
                h0h = apool.tile([128, 2], F32, tag="h0h")
                for hh in range(2):
                    sl = slice(hh * HH2, (hh + 1) * HH2)
                    nc.vector.tensor_scalar(scrD[:, sl], sm_bf[:, sl], M_ap[:],
                                            -QMIN, ALU.subtract, ALU.min)
                    nc.scalar.activation(lms[:, sl], scrD[:, sl], AF.Ln,
                                         scale=-1.0)
                    nc.vector.tensor_mul(d_sb[:, sl], scrD[:, sl], lms[:, sl])
                    nc.vector.tensor_scalar(d_sb[:, sl], d_sb[:, sl], 1.0, 0.0,
                                            ALU.mult, ALU.add,
                                            accum_out=h0h[:, hh : hh + 1])
                nc.vector.tensor_scalar(accs["acc_h0"][:], h0h[:, 0:1], 1.0,
                                        h0h[:, 1:2], ALU.mult, ALU.add)

            # ---------------- outputs ----------------
            for n in out_names:
                nc.sync.dma_start(out=outs[n], in_=accs[n][:])

    nc.compile()
    meta = dict(B=B, H=H, W=W, NCORES=NCORES, HL=HL, CH=CH, G=G, FD=FD,
                out_names=out_names)
    return nc, meta


# ---------------------------------------------------------------------------
_CACHE = {}


def _get_built(key=(8, 1024, 1024, 8)):
    if key not in _CACHE:
        _CACHE[key] = build_kernel(*key)
    return _CACHE[key]


def _to_tiles(slab):
    """[8, HH, W] -> [128, (HH/16)*W] with partition p = r*8+b, free (t, w)."""
    B, HH, W = slab.shape
    T = HH // 16
    a = slab.reshape(B, T, 16, W).transpose(2, 0, 1, 3)  # [16, 8, T, W]
    return np.ascontiguousarray(a).reshape(128, T * W)


def make_in_maps(target, pred, spred, NCORES, HL):
    B = target.shape[0]
    H, W = target.shape[-2], target.shape[-1]
    t2 = np.asarray(target, dtype=np.float32).reshape(B, H, W).astype(np.float16)
    tpad = np.pad(t2, ((0, 0), (R, R), (0, 0)), mode="symmetric")
    p2 = np.asarray(pred, dtype=np.float32).reshape(B, H, W).astype(np.float16)
    s2 = np.asarray(spred, dtype=np.float32).reshape(B, H, W).astype(np.float16)
    in_maps = []
    for i in range(NCORES):
        in_maps.append({
            "targ": _to_tiles(tpad[:, i * HL : i * HL + HL + 2 * R, :]),
            "tstd": _to_tiles(t2[:, i * HL : (i + 1) * HL, :]),
            "pred": _to_tiles(p2[:, i * HL : (i + 1) * HL, :]),
            "spred": _to_tiles(s2[:, i * HL : (i + 1) * HL, :]),
        })
    return in_maps


def host_t_sums(target):
    """Exact per-batch sums of the binary target (host side, fp64)."""
    B = target.shape[0]
    return np.asarray(target, dtype=np.float64).reshape(B, -1).sum(axis=1)


def assemble(results, meta, n_tot, t_b, return_parts=False):
    out_names = meta["out_names"]
    NC = meta["NCORES"]
    packed = np.stack([np.asarray(results[i]["outp"], dtype=np.float64)
                       for i in range(NC)])  # [NC, 128, 10+CH]
    acc = {n: packed[:, :, i] for i, n in enumerate(out_names)}
    ssum = packed[:, :, len(out_names):].reshape(NC, -1)

    per_b = lambda a: a.reshape(NC, 16, 8).sum(axis=(0, 1))  # noqa: E731
    tot = lambda a: float(a.sum())  # noqa: E731

    pt_b = per_b(acc["acc_pt"])
    pp_b = per_b(acc["acc_pp"])
    S_sp = -tot(acc["acc_sp"])  # device accumulates ln(1-p) = -softplus(x)
    S_xt = tot(acc["acc_xt"])
    S_dd = tot(acc["acc_dd"])
    S_sp2 = tot(acc["acc_sp2"])
    S_smp = tot(acc["acc_smp"])
    S_smlog = tot(acc["acc_smlog"])
    S_h0 = tot(acc["acc_h0"])
    S_sm = float(ssum.sum())
    t_cnt = float(t_b.sum())

    mx = float(acc["acc_max"].max())
    M = float(np.float32(mx) + np.float32(EPS8))

    dice = float(np.mean(1.0 - (2.0 * pt_b + SMOOTH) / (pp_b + t_b + SMOOTH)))
    bce = (S_sp - S_xt) / n_tot

    if mx < 1e-8:
        kl = 0.0
    else:
        lnM = np.log(M)
        sum_t1_ln = (S_smlog - lnM * S_sm) / M
        sum_t0_ln = ((-S_h0) - lnM * (n_tot * M - S_sm)) / M
        sum_t1u = (S_sm - 2.0 * S_smp) / M
        kl = (sum_t1_ln + sum_t0_ln + S_sp2 + sum_t1u) / n_tot
        kl = min(max(kl, 0.0), 2.0)

    mxm = 1.0 if t_cnt > 0 else 0.0
    mnm = 0.0 if t_cnt < n_tot else 1.0
    if mnm == 1.0 or mxm == 0.0:
        S_xts = 0.0
    else:
        S_xts = S_xt / (mxm - mnm + float(np.float32(1e-8)))
    bsoft = (S_sp - S_xts) / n_tot

    div = S_dd / n_tot

    lam = np.array([1.0, 1.0, 0.5, 0.5, 0.5])
    lam = lam / lam.sum()
    out = lam[0] * dice + lam[1] * bce + lam[2] * kl + lam[3] * div + lam[4] * bsoft
    if return_parts:
        return np.float32(out), dict(dice=dice, bce=bce, kl=kl, div=div,
                                     bsoft=bsoft, mx=mx, S_sm=S_sm, S_smp=S_smp,
                                     S_smlog=S_smlog, S_h0=-S_h0, S_sp=S_sp,
                                     S_sp2=S_sp2, S_xt=S_xt, S_dd=S_dd)
    return np.asarray(out, dtype=np.float32).reshape(())


def kernel(image, pred, target, second_pred):
    nc, meta = _get_built()
    in_maps = make_in_maps(target, pred, second_pred, meta["NCORES"], meta["HL"])
    res = run_bass_kernel_spmd(nc, in_maps, core_ids=list(range(meta["NCORES"])))
    n_tot = float(np.prod(target.shape))
    return assemble(res.results, meta, n_tot, host_t_sums(target))

